# revision 2
# baseline (speedup 1.0000x reference)
"""DARTS-cell (moe_routing) Trainium2 kernel, v2.

Data-parallel over batch B=32 across 8 cores (4 samples/core).

v2 changes vs v1:
- bf16 for all depthwise work: DVE taps run in 2x_1P mode (alignment
  handled via +1-column twin buffers for odd-offset taps), PE-fused 5x5
  tap matrices are bf16 (halves fw DMA to ~137 MB/core).
- shared relu z-buffer per (state, sample): alpha refolded into stage-2
  activations (sep3/sep5), per-sample dil3 tap tables, and the dil5
  z40 build, so the first-stage relu is written once, not per-branch.
- emission software-pipelined: per-j PE stream (sep5s1 -> pw_s3s1 ->
  dil5 -> pw_d3 -> sep5s2 -> pw_s3s2), next sample's preprocess emitted
  during the last step to keep the PE warm across sample boundaries.
- tunable dil5 PE<->DVE split (DIL5_DVE) for engine balancing.
"""

import sys

sys.path.insert(0, "/opt/trn_rl_repo")

import numpy as np
import ml_dtypes
from concourse import bacc, mybir, tile
from concourse.bass_utils import run_bass_kernel_spmd

STEPS = 4
N_MIX = 14
OFFSETS = [0, 2, 5, 9]
B, C_IN, C, H, W = 32, 512, 128, 32, 32
HW = H * W
N_CORES = 8
BL = B // N_CORES
BN_SCALE = float(1.0 / np.sqrt(1.0 + 1e-5))

F32 = mybir.dt.float32
F32R = mybir.dt.float32r
BF16 = mybir.dt.bfloat16
ALU = mybir.AluOpType
ACTF = mybir.ActivationFunctionType

O_MAX, O_AVG, O_SKIP, O_SEP3, O_SEP5, O_DIL3, O_DIL5 = 1, 2, 3, 4, 5, 6, 7

# dwt layout per m: [s3a(9), s3b(9), d5(25)]
TAP_S3A, TAP_S3B, TAP_D5 = 0, 9, 18
N_TAPS = 43
# pw slots: s3a, s3b, d3, d5
PW_S3A, PW_S3B, PW_D3, PW_D5 = 0, 1, 2, 3
N_PW = 4

# (m, b) pairs whose dil5 branch runs on DVE instead of PE-fused.
# Spread across samples and mixed-ops for schedule smoothness.
DIL5_DVE_COUNT = 24
_pairs = [(m, b) for b in range(BL) for m in range(N_MIX)]
DIL5_DVE = set(_pairs[i] for i in
               np.linspace(0, len(_pairs) - 1, DIL5_DVE_COUNT).astype(int)) \
    if DIL5_DVE_COUNT else set()


def _host_alphas(gates, top):
    g = gates.astype(np.float64)
    idx = np.argsort(-g, axis=-1, kind="stable")[..., :top]
    mask = np.zeros(g.shape, bool)
    np.put_along_axis(mask, idx, True, axis=-1)
    gm = np.where(mask, g, -np.inf)
    gm -= gm.max(axis=-1, keepdims=True)
    e = np.exp(gm)
    p = e / e.sum(axis=-1, keepdims=True)
    return p.astype(np.float32)


def build_program(n_cores=N_CORES):
    nc = bacc.Bacc("TRN2", target_bir_lowering=False, debug=False,
                   num_devices=n_cores)

    x0_d = nc.dram_tensor("x0", [BL, 4, 128, HW], F32, kind="ExternalInput").ap()
    x1_d = nc.dram_tensor("x1", [BL, 4, 128, HW], F32, kind="ExternalInput").ap()
    prew_d = nc.dram_tensor("prew", [128, 2, 4, 128], F32R, kind="ExternalInput").ap()
    pw_d = nc.dram_tensor("pw", [128, N_MIX, N_PW, 128], BF16, kind="ExternalInput").ap()
    fw5a_d = nc.dram_tensor("fw5a", [128, N_MIX, 25, 128], BF16, kind="ExternalInput").ap()
    fw5b_d = nc.dram_tensor("fw5b", [128, N_MIX, 25, 128], BF16, kind="ExternalInput").ap()
    fwd5_d = nc.dram_tensor("fwd5", [128, N_MIX, 25, 128], BF16, kind="ExternalInput").ap()
    dwt_d = nc.dram_tensor("dwt", [128, N_MIX, N_TAPS], F32, kind="ExternalInput").ap()
    dwd3_d = nc.dram_tensor("dwd3", [128, N_MIX, BL, 9], F32, kind="ExternalInput").ap()
    alf_d = nc.dram_tensor("alf", [128, N_MIX, BL, 8], F32, kind="ExternalInput").ap()
    rmap_d = nc.dram_tensor("rmap", [128, 32, 32], F32, kind="ExternalInput").ap()
    out_d = nc.dram_tensor("out", [BL, 4, 128, HW], F32, kind="ExternalOutput").ap()

    with tile.TileContext(nc) as tc:
        with (
            tc.tile_pool(name="const", bufs=1) as cpool,
            tc.tile_pool(name="work", bufs=1) as wpool,
            tc.tile_pool(name="xs", bufs=2) as xpool,
            tc.tile_pool(name="dwa", bufs=3) as dpool,
            tc.tile_pool(name="ps_state", bufs=2, space="PSUM") as pspool,
            tc.tile_pool(name="ps_scr", bufs=2, space="PSUM") as scrpool,
            tc.tile_pool(name="fw", bufs=2) as fwpool,
        ):
            # ---- constants / weights ----
            prew = cpool.tile([128, 2, 4, 128], F32R, tag="prew")
            pw = cpool.tile([128, N_MIX, N_PW, 128], BF16, tag="pw")
            dwt = cpool.tile([128, N_MIX, N_TAPS], F32, tag="dwt")
            dwd3 = cpool.tile([128, N_MIX, BL, 9], F32, tag="dwd3")
            alf = cpool.tile([128, N_MIX, BL, 8], F32, tag="alf")
            rmap = cpool.tile([128, 32, 32], F32, tag="rmap")
            nc.sync.dma_start(prew[:], prew_d)
            nc.sync.dma_start(pw[:], pw_d)
            nc.sync.dma_start(dwt[:], dwt_d)
            nc.sync.dma_start(dwd3[:], dwd3_d)
            nc.sync.dma_start(alf[:], alf_d)
            nc.sync.dma_start(rmap[:], rmap_d)

            # ---- persistent padded work buffers (bf16, zero borders) ----
            z36 = [wpool.tile([128, 36, 36], BF16, tag=f"z36_{i}", name=f"z36_{i}") for i in range(2)]
            z36t = [wpool.tile([128, 36, 36], BF16, tag=f"z36t_{i}", name=f"z36t_{i}") for i in range(2)]
            z2b = [wpool.tile([128, 36, 36], BF16, tag=f"z2b_{i}", name=f"z2b_{i}") for i in range(2)]
            z2bt = [wpool.tile([128, 36, 36], BF16, tag=f"z2bt_{i}", name=f"z2bt_{i}") for i in range(2)]
            z5b = [wpool.tile([128, 36, 36], BF16, tag=f"z5b_{i}", name=f"z5b_{i}") for i in range(2)]
            z40 = [wpool.tile([128, 40, 40], BF16, tag=f"z40_{i}", name=f"z40_{i}") for i in range(2)]
            xpmax = wpool.tile([128, 34, 34], F32, tag="xpmax")
            xpsum = wpool.tile([128, 34, 34], F32, tag="xpsum")
            rmpad = wpool.tile([128, 34, 32], F32, tag="rmpad")
            rspad = wpool.tile([128, 34, 32], F32, tag="rspad")
            ptmp = [wpool.tile([128, 32, 32], F32, tag=f"ptmp_{i}", name=f"ptmp_{i}") for i in range(2)]

            states_t = [wpool.tile([128, 6, 32, 32], F32, tag=f"states_{i}", name=f"states_{i}") for i in range(2)]
            pooled = wpool.tile([128, 2, 5, 32, 32], BF16, tag="pooled")

            for z in z36 + z36t + z2b + z2bt + z5b + z40:
                nc.gpsimd.memset(z[:], 0.0)
            nc.gpsimd.memset(xpmax[:], -1e30)
            nc.gpsimd.memset(xpsum[:], 0.0)
            nc.gpsimd.memset(rmpad[:], -1e30)
            nc.gpsimd.memset(rspad[:], 0.0)

            def flat(ap3):
                return ap3.rearrange("p a b -> p (a b)")

            def mm_chunks(psum3, lhsT, rhs3, fl):
                """two N=512 matmuls; fl(chunk) -> (start, stop)."""
                for h in range(2):
                    s, e = fl(h)
                    nc.tensor.matmul(psum3[:, 16 * h:16 * h + 16, :], lhsT,
                                     rhs3[:, 16 * h:16 * h + 16, :],
                                     start=s, stop=e)

            def dw_chain(z, zt, dwacc, sc_of, k, pad, stride, interior):
                """dwacc = sum_t sc(t) * shift_t(z); zt = +1-col twin for
                odd x-offsets (None if all offsets even)."""
                first = True
                for ky in range(k):
                    for kx in range(k):
                        t = ky * k + kx
                        y0 = interior - pad + stride * ky
                        x0 = interior - pad + stride * kx
                        if x0 % 2 == 1:
                            assert zt is not None
                            view = zt[:, y0:y0 + 32, x0 - 1:x0 + 31]
                        else:
                            view = z[:, y0:y0 + 32, x0:x0 + 32]
                        sc = sc_of(t)
                        if first:
                            nc.vector.tensor_scalar_mul(dwacc[:], view, sc)
                            first = False
                        else:
                            nc.vector.scalar_tensor_tensor(
                                dwacc[:], view, sc, dwacc[:],
                                op0=ALU.mult, op1=ALU.add)

            def fused_stage(fw_tile, zt, pad, stride, interior, psum3, fl):
                """25 fused dw+pw tap matmuls accumulating into psum3."""
                for t in range(25):
                    ky, kx = divmod(t, 5)
                    y0 = interior - pad + stride * ky
                    x0 = interior - pad + stride * kx
                    for h in range(2):
                        s, e = fl(t, h)
                        nc.tensor.matmul(
                            psum3[:, 16 * h:16 * h + 16, :],
                            fw_tile[:, t, :],
                            zt[:, y0 + 16 * h:y0 + 16 * h + 16, x0:x0 + 32],
                            start=s, stop=e)

            class StpFlags:
                """start/stop flags per 16-row chunk of the step PSUM."""
                def __init__(self, total):
                    self.total = total
                    self.idx = [0, 0]

                def next(self, h):
                    i = self.idx[h]
                    self.idx[h] += 1
                    return (i == 0, i == self.total - 1)

            def preproc(b):
                states = states_t[b % 2]
                for inp, xd in ((0, x0_d), (1, x1_d)):
                    scr = pspool.tile([128, 32, 32], F32, tag="stp")
                    for kc in range(4):
                        xb = xpool.tile([128, HW], F32, tag="xb")
                        nc.sync.dma_start(xb[:], xd[b, kc])
                        xr = xpool.tile([128, HW], F32R, tag="xr")
                        nc.scalar.activation(xr[:], xb[:], ACTF.Relu)
                        for h in range(2):
                            nc.tensor.matmul(
                                scr[:, 16 * h:16 * (h + 1), :],
                                prew[:, inp, kc, :],
                                xr[:, 512 * h:512 * (h + 1)].rearrange(
                                    "p (a c) -> p a c", a=16),
                                start=(kc == 0), stop=(kc == 3))
                    nc.scalar.copy(states[:, inp], scr[:])

            def build_pools(j, b):
                states = states_t[b % 2]
                x3 = states[:, j]
                nc.scalar.copy(xpmax[:, 1:33, 1:33], x3)
                t = ptmp[0]
                nc.vector.tensor_max(t[:], xpmax[:, 1:33, 0:32],
                                     xpmax[:, 1:33, 1:33])
                nc.vector.tensor_max(rmpad[:, 1:33, :], t[:],
                                     xpmax[:, 1:33, 2:34])
                nc.vector.tensor_max(t[:], rmpad[:, 0:32, :],
                                     rmpad[:, 1:33, :])
                nc.vector.tensor_max(pooled[:, 0, j], t[:],
                                     rmpad[:, 2:34, :])
                nc.scalar.copy(xpsum[:, 1:33, 1:33], x3)
                t = ptmp[1]
                nc.gpsimd.tensor_add(t[:], xpsum[:, 1:33, 0:32],
                                     xpsum[:, 1:33, 1:33])
                nc.gpsimd.tensor_add(rspad[:, 1:33, :], t[:],
                                     xpsum[:, 1:33, 2:34])
                nc.gpsimd.tensor_add(t[:], rspad[:, 0:32, :],
                                     rspad[:, 1:33, :])
                nc.gpsimd.tensor_add(ptmp[0][:], t[:], rspad[:, 2:34, :])
                nc.gpsimd.tensor_mul(pooled[:, 1, j], ptmp[0][:], rmap[:])

            # ================= main =================
            preproc(0)
            for b in range(BL):
                states = states_t[b % 2]
                for step in range(STEPS):
                    n_in = 2 + step
                    m0 = OFFSETS[step]
                    stp = pspool.tile([128, 32, 32], F32, tag="stp")
                    # stp matmuls per chunk: per j: dil5(25 PE | 1 DVE-pw)
                    # + sep5s2(25) + pw_s3s2(1) + pw_d3(1)
                    total = 0
                    for j in range(n_in):
                        m = m0 + j
                        total += (25 if (m, b) not in DIL5_DVE else 1) + 25 + 1 + 1
                    fl = StpFlags(total)

                    # deferred stage-2 emitters, one j behind
                    pending = []

                    def emit_pending():
                        while pending:
                            pending.pop(0)()

                    for j in range(n_in):
                        m = m0 + j
                        jb = j % 2
                        # ---- shared relu z build (ScalarE) ----
                        nc.scalar.activation(z36[jb][:, 2:34, 2:34],
                                             states[:, j], ACTF.Relu)
                        nc.scalar.activation(z36t[jb][:, 2:34, 1:33],
                                             states[:, j], ACTF.Relu)
                        # ---- sep5 stage 1 (PE) ----
                        fw_a = fwpool.tile([128, 25, 128], BF16, tag="fw")
                        nc.sync.dma_start(fw_a[:], fw5a_d[:, m])
                        scr5 = scrpool.tile([128, 32, 32], F32, tag="scr")
                        fused_stage(fw_a, z36[jb], 2, 1, 2, scr5,
                                    lambda t, h: (t == 0, t == 24))
                        # ---- sep3 stage 1 (DVE + PE) ----
                        da1 = dpool.tile([128, 32, 32], BF16, tag="dwacc")
                        dw_chain(z36[jb], z36t[jb], da1,
                                 lambda t: dwt[:, m, TAP_S3A + t:TAP_S3A + t + 1],
                                 3, 1, 1, 2)
                        scr3 = scrpool.tile([128, 32, 32], F32, tag="scr")
                        mm_chunks(scr3, pw[:, m, PW_S3A, :], da1,
                                  lambda h: (True, True))
                        # ---- dil5 (PE-fused or DVE) ----
                        if (m, b) not in DIL5_DVE:
                            nc.scalar.activation(z40[jb][:, 4:36, 4:36],
                                                 states[:, j], ACTF.Relu,
                                                 scale=alf[:, m, b, O_DIL5:O_DIL5 + 1])
                            fw_d = fwpool.tile([128, 25, 128], BF16, tag="fw")
                            nc.sync.dma_start(fw_d[:], fwd5_d[:, m])
                            fused_stage(fw_d, z40[jb], 4, 2, 4, stp,
                                        lambda t, h: fl.next(h))
                        else:
                            nc.scalar.activation(z40[jb][:, 4:36, 4:36],
                                                 states[:, j], ACTF.Relu,
                                                 scale=alf[:, m, b, O_DIL5:O_DIL5 + 1])
                            da5 = dpool.tile([128, 32, 32], BF16, tag="dwacc")
                            dw_chain(z40[jb], None, da5,
                                     lambda t: dwt[:, m, TAP_D5 + t:TAP_D5 + t + 1],
                                     5, 4, 2, 4)
                            mm_chunks(stp, pw[:, m, PW_D5, :], da5,
                                      lambda h: fl.next(h))
                        # ---- dil3 (DVE + PE) ----
                        da3 = dpool.tile([128, 32, 32], BF16, tag="dwacc")
                        dw_chain(z36[jb], None, da3,
                                 lambda t: dwd3[:, m, b, t:t + 1],
                                 3, 2, 2, 2)
                        mm_chunks(stp, pw[:, m, PW_D3, :], da3,
                                  lambda h: fl.next(h))

                        # ---- stage 2s (deferred to overlap round trips) ----
                        def make_stage2(m=m, jb=jb, scr5=scr5, scr3=scr3):
                            def emit():
                                # sep5 stage 2
                                nc.scalar.activation(
                                    z5b[jb][:, 2:34, 2:34], scr5[:], ACTF.Relu,
                                    scale=alf[:, m, b, O_SEP5:O_SEP5 + 1])
                                fw_b = fwpool.tile([128, 25, 128], BF16, tag="fw")
                                nc.sync.dma_start(fw_b[:], fw5b_d[:, m])
                                fused_stage(fw_b, z5b[jb], 2, 1, 2, stp,
                                            lambda t, h: fl.next(h))
                                # sep3 stage 2
                                nc.scalar.activation(
                                    z2b[jb][:, 2:34, 2:34], scr3[:], ACTF.Relu,
                                    scale=alf[:, m, b, O_SEP3:O_SEP3 + 1])
                                nc.scalar.activation(
                                    z2bt[jb][:, 2:34, 1:33], scr3[:], ACTF.Relu,
                                    scale=alf[:, m, b, O_SEP3:O_SEP3 + 1])
                                da2 = dpool.tile([128, 32, 32], BF16, tag="dwacc")
                                dw_chain(z2b[jb], z2bt[jb], da2,
                                         lambda t: dwt[:, m, TAP_S3B + t:TAP_S3B + t + 1],
                                         3, 1, 1, 2)
                                mm_chunks(stp, pw[:, m, PW_S3B, :], da2,
                                          lambda h: fl.next(h))
                            return emit

                        pending.append(make_stage2())
                        if len(pending) > 1:
                            pending.pop(0)()

                    emit_pending()

                    # ---- pools for new states ----
                    if step == 0:
                        build_pools(0, b)
                        build_pools(1, b)
                    else:
                        build_pools(1 + step, b)

                    # ---- pool/skip accumulation (post matmuls) ----
                    for j in range(n_in):
                        m = m0 + j
                        for (src, o) in ((pooled[:, 0, j], O_MAX),
                                         (pooled[:, 1, j], O_AVG),
                                         (states[:, j], O_SKIP)):
                            nc.vector.scalar_tensor_tensor(
                                stp[:], src, alf[:, m, b, o:o + 1], stp[:],
                                op0=ALU.mult, op1=ALU.add)

                    # ---- evacuate state ----
                    nc.scalar.copy(states[:, 2 + step], stp[:])

                    if step == STEPS - 1 and b + 1 < BL:
                        preproc(b + 1)

                # ---- output ----
                for i in range(4):
                    nc.sync.dma_start(out_d[b, i], flat(states[:, 2 + i]))

    nc.compile()
    return nc


def host_prepare(inputs):
    s0, s1 = np.asarray(inputs["s0"]), np.asarray(inputs["s1"])
    gates = np.asarray(inputs["gates"])
    top = int(inputs["top"])
    p = _host_alphas(gates, top)  # [N_MIX, B, 8]

    prew = np.empty((128, 2, 4, 128), np.float32)
    for inp, wname in ((0, "pre0_w"), (1, "pre1_w")):
        wmat = np.asarray(inputs[wname]) * BN_SCALE
        for kc in range(4):
            prew[:, inp, kc, :] = wmat[:, 128 * kc:128 * (kc + 1)].T

    pw = np.empty((128, N_MIX, N_PW, 128), np.float32)
    for slot, key in ((PW_S3A, "sep3_pw1"), (PW_S3B, "sep3_pw2"),
                      (PW_D3, "dil3_pw"), (PW_D5, "dil5_pw")):
        wmat = np.asarray(inputs[key]).astype(np.float32) * BN_SCALE
        pw[:, :, slot, :] = wmat.transpose(2, 0, 1)

    def fuse(pw_key, dw_key):
        pwm = np.asarray(inputs[pw_key]).astype(np.float32) * BN_SCALE
        dwm = np.asarray(inputs[dw_key]).astype(np.float32).reshape(N_MIX, C, 25)
        pwT = pwm.transpose(2, 0, 1)
        dwT = dwm.transpose(1, 0, 2)
        return (pwT[:, :, None, :] * dwT[:, :, :, None]).astype(ml_dtypes.bfloat16)

    fw5a = fuse("sep5_pw1", "sep5_dw1")
    fw5b = fuse("sep5_pw2", "sep5_dw2")
    fwd5 = fuse("dil5_pw", "dil5_dw")

    dwt = np.empty((128, N_MIX, N_TAPS), np.float32)
    for t0, key, k in ((TAP_S3A, "sep3_dw1", 3), (TAP_S3B, "sep3_dw2", 3),
                       (TAP_D5, "dil5_dw", 5)):
        w = np.asarray(inputs[key])
        dwt[:, :, t0:t0 + k * k] = (
            w.reshape(N_MIX, C, k * k).transpose(1, 0, 2))

    # avg-pool BN/count map
    cnt = np.zeros((32, 32), np.float32)
    for dy in (-1, 0, 1):
        for dx in (-1, 0, 1):
            cnt[max(0, dy):32 - max(0, -dy),
                max(0, dx):32 - max(0, -dx)] += 1
    rmap = np.broadcast_to((BN_SCALE / cnt).astype(np.float32),
                           (128, 32, 32)).copy()

    d3w = np.asarray(inputs["dil3_dw"]).astype(np.float32).reshape(N_MIX, C, 9)

    in_maps = []
    for core in range(N_CORES):
        sl = slice(core * BL, (core + 1) * BL)
        alf = p[:, sl, :].copy()  # [N_MIX, BL, 8]
        alf[:, :, O_MAX] *= BN_SCALE
        alf_b = np.broadcast_to(alf, (128,) + alf.shape).copy()
        # dil3 taps with per-sample alpha folded in
        dwd3 = np.einsum('mct,mb->cmbt', d3w, p[:, sl, O_DIL3]).astype(np.float32)
        dwd3 = np.ascontiguousarray(dwd3)
        in_maps.append({
            "x0": s0[sl].reshape(BL, 4, 128, HW).astype(np.float32),
            "x1": s1[sl].reshape(BL, 4, 128, HW).astype(np.float32),
            "prew": prew,
            "pw": pw.astype(ml_dtypes.bfloat16),
            "dwt": dwt, "dwd3": dwd3,
            "fw5a": fw5a, "fw5b": fw5b, "fwd5": fwd5,
            "alf": alf_b.astype(np.float32), "rmap": rmap,
        })
    return in_maps, p


_prog_cache = {}


def _get_dense_program():
    if "v2" not in _prog_cache:
        _prog_cache["v2"] = build_program()
    return _prog_cache["v2"]


def kernel(**inputs):
    in_maps, _ = host_prepare(inputs)
    nc = _get_dense_program()
    res = run_bass_kernel_spmd(nc, in_maps, core_ids=list(range(N_CORES)))
    out = np.empty((B, 512, H, W), np.float32)
    for core in range(N_CORES):
        o = res.results[core]["out"]
        out[core * BL:(core + 1) * BL] = (
            o.reshape(BL, 512, H, W).astype(np.float32))
    return out


# revision 3
# speedup vs baseline: 1.0030x; 1.0030x over previous
"""DARTS-cell (moe_routing) Trainium2 kernel, v3.

Data-parallel over batch B=32 across 8 cores (4 samples/core).

Engine split (measured rates on TRN2):
- PE (fused dw+pw tap matmuls, ~244 ns/MM): sep5 both stages, dil5 for
  most (m,b), all 1x1 convs, preprocess.
- DVE: sep3 tap chains (tensor_scalar first tap ~476 ns, STT ~1285 ns
  each - STT has no fast mode on cayman), dil5 chains for a few (m,b),
  pool maxes, pool/skip accumulation into state PSUM.
- ScalarE + GpSimd hybrid: dil3 tap chains (ScalarE per-partition mul
  into contiguous partials ~1.15 us, GpSimd contiguous adds ~0.6-1.4 us)
  plus relu/pad builds (Sc) and avg pools (GpSimd).

alpha folding: sep3/sep5 into stage-2 relu scale, dil3 into per-sample
host tap tables, dil5 into the z40 build; pools/skip via STT scalars.
"""

import sys

sys.path.insert(0, "/opt/trn_rl_repo")

import numpy as np
import ml_dtypes
from concourse import bacc, mybir, tile
from concourse.bass_utils import run_bass_kernel_spmd

STEPS = 4
N_MIX = 14
OFFSETS = [0, 2, 5, 9]
B, C_IN, C, H, W = 32, 512, 128, 32, 32
HW = H * W
N_CORES = 8
BL = B // N_CORES
BN_SCALE = float(1.0 / np.sqrt(1.0 + 1e-5))

F32 = mybir.dt.float32
F32R = mybir.dt.float32r
BF16 = mybir.dt.bfloat16
ALU = mybir.AluOpType
ACTF = mybir.ActivationFunctionType

O_MAX, O_AVG, O_SKIP, O_SEP3, O_SEP5, O_DIL3, O_DIL5 = 1, 2, 3, 4, 5, 6, 7

TAP_S3A, TAP_S3B, TAP_D5 = 0, 9, 18
N_TAPS = 43
PW_S3A, PW_S3B, PW_D3, PW_D5 = 0, 1, 2, 3
N_PW = 4

# (m, b) pairs whose dil5 branch runs as a pure-DVE tap chain (PE relief)
DIL5_DVE_COUNT = 14
_pairs = [(m, b) for b in range(BL) for m in range(N_MIX)]
DIL5_DVE = set(_pairs[i] for i in
               np.linspace(0, len(_pairs) - 1, DIL5_DVE_COUNT).astype(int)) \
    if DIL5_DVE_COUNT else set()


def _host_alphas(gates, top):
    g = gates.astype(np.float64)
    idx = np.argsort(-g, axis=-1, kind="stable")[..., :top]
    mask = np.zeros(g.shape, bool)
    np.put_along_axis(mask, idx, True, axis=-1)
    gm = np.where(mask, g, -np.inf)
    gm -= gm.max(axis=-1, keepdims=True)
    e = np.exp(gm)
    p = e / e.sum(axis=-1, keepdims=True)
    return p.astype(np.float32)


def build_program(n_cores=N_CORES):
    nc = bacc.Bacc("TRN2", target_bir_lowering=False, debug=False,
                   num_devices=n_cores)

    x0_d = nc.dram_tensor("x0", [BL, 4, 128, HW], F32, kind="ExternalInput").ap()
    x1_d = nc.dram_tensor("x1", [BL, 4, 128, HW], F32, kind="ExternalInput").ap()
    prew_d = nc.dram_tensor("prew", [128, 2, 4, 128], F32R, kind="ExternalInput").ap()
    pw_d = nc.dram_tensor("pw", [128, N_MIX, N_PW, 128], BF16, kind="ExternalInput").ap()
    fw5a_d = nc.dram_tensor("fw5a", [128, N_MIX, 25, 128], BF16, kind="ExternalInput").ap()
    fw5b_d = nc.dram_tensor("fw5b", [128, N_MIX, 25, 128], BF16, kind="ExternalInput").ap()
    fwd5_d = nc.dram_tensor("fwd5", [128, N_MIX, 25, 128], BF16, kind="ExternalInput").ap()
    dwt_d = nc.dram_tensor("dwt", [128, N_MIX, N_TAPS], F32, kind="ExternalInput").ap()
    dwd3_d = nc.dram_tensor("dwd3", [128, N_MIX, BL, 9], F32, kind="ExternalInput").ap()
    alf_d = nc.dram_tensor("alf", [128, N_MIX, BL, 8], F32, kind="ExternalInput").ap()
    rmap_d = nc.dram_tensor("rmap", [128, 32, 32], F32, kind="ExternalInput").ap()
    out_d = nc.dram_tensor("out", [BL, 4, 128, HW], F32, kind="ExternalOutput").ap()

    with tile.TileContext(nc) as tc:
        with (
            tc.tile_pool(name="const", bufs=1) as cpool,
            tc.tile_pool(name="work", bufs=1) as wpool,
            tc.tile_pool(name="xs", bufs=2) as xpool,
            tc.tile_pool(name="dwa", bufs=4) as dpool,
            tc.tile_pool(name="hyb", bufs=4) as hpool,
            tc.tile_pool(name="ps_state", bufs=2, space="PSUM") as pspool,
            tc.tile_pool(name="ps_scr", bufs=2, space="PSUM") as scrpool,
            tc.tile_pool(name="fw", bufs=2) as fwpool,
        ):
            prew = cpool.tile([128, 2, 4, 128], F32R, tag="prew")
            pw = cpool.tile([128, N_MIX, N_PW, 128], BF16, tag="pw")
            dwt = cpool.tile([128, N_MIX, N_TAPS], F32, tag="dwt")
            dwd3 = cpool.tile([128, N_MIX, BL, 9], F32, tag="dwd3")
            alf = cpool.tile([128, N_MIX, BL, 8], F32, tag="alf")
            rmap = cpool.tile([128, 32, 32], F32, tag="rmap")
            nc.sync.dma_start(prew[:], prew_d)
            nc.sync.dma_start(pw[:], pw_d)
            nc.sync.dma_start(dwt[:], dwt_d)
            nc.sync.dma_start(dwd3[:], dwd3_d)
            nc.sync.dma_start(alf[:], alf_d)
            nc.sync.dma_start(rmap[:], rmap_d)

            z36 = [wpool.tile([128, 36, 36], BF16, tag=f"z36_{i}", name=f"z36_{i}") for i in range(2)]
            z2b = [wpool.tile([128, 36, 36], BF16, tag=f"z2b_{i}", name=f"z2b_{i}") for i in range(2)]
            z5b = [wpool.tile([128, 36, 36], BF16, tag=f"z5b_{i}", name=f"z5b_{i}") for i in range(2)]
            z40 = [wpool.tile([128, 40, 40], BF16, tag=f"z40_{i}", name=f"z40_{i}") for i in range(2)]
            xpmax = wpool.tile([128, 34, 34], F32, tag="xpmax")
            xpsum = wpool.tile([128, 34, 34], F32, tag="xpsum")
            rmpad = wpool.tile([128, 34, 32], F32, tag="rmpad")
            rspad = wpool.tile([128, 34, 32], F32, tag="rspad")
            ptmp = [wpool.tile([128, 32, 32], F32, tag=f"ptmp_{i}", name=f"ptmp_{i}") for i in range(2)]

            states_t = [wpool.tile([128, 6, 32, 32], F32, tag=f"states_{i}", name=f"states_{i}") for i in range(2)]
            pooled = wpool.tile([128, 2, 5, 32, 32], BF16, tag="pooled")

            for z in z36 + z2b + z5b + z40:
                nc.gpsimd.memset(z[:], 0.0)
            nc.gpsimd.memset(xpmax[:], -1e30)
            nc.gpsimd.memset(xpsum[:], 0.0)
            nc.gpsimd.memset(rmpad[:], -1e30)
            nc.gpsimd.memset(rspad[:], 0.0)

            def flat(ap3):
                return ap3.rearrange("p a b -> p (a b)")

            def mm_chunks(psum3, lhsT, rhs3, fl):
                for h in range(2):
                    s, e = fl(h)
                    nc.tensor.matmul(psum3[:, 16 * h:16 * h + 16, :], lhsT,
                                     rhs3[:, 16 * h:16 * h + 16, :],
                                     start=s, stop=e)

            def tap_views(z, k, pad, stride, interior):
                out = []
                for ky in range(k):
                    for kx in range(k):
                        t = ky * k + kx
                        y0 = interior - pad + stride * ky
                        x0 = interior - pad + stride * kx
                        out.append((t, x0 % 2 == 0,
                                    z[:, y0:y0 + 32, x0:x0 + 32]))
                # an even-x0 tap first: tensor_scalar (4x) leads the chain
                out.sort(key=lambda e: (not e[1],))
                return out

            def dw_chain_dve(z, dwacc, sc_of, k, pad, stride, interior):
                for i, (t, _, view) in enumerate(tap_views(z, k, pad, stride, interior)):
                    if i == 0:
                        nc.vector.tensor_scalar_mul(dwacc[:], view, sc_of(t))
                    else:
                        nc.vector.scalar_tensor_tensor(
                            dwacc[:], view, sc_of(t), dwacc[:],
                            op0=ALU.mult, op1=ALU.add)

            def dw_chain_hyb(z, acc, sc_of, k, pad, stride, interior):
                """partials on ScalarE, accumulation on GpSimd (contiguous)."""
                prev = None
                started = False
                for t, _, view in tap_views(z, k, pad, stride, interior):
                    tmp = hpool.tile([128, 32, 32], BF16, tag="hp")
                    nc.scalar.mul(tmp[:], view, sc_of(t))
                    if prev is None and not started:
                        prev = tmp
                    elif not started:
                        nc.gpsimd.tensor_add(flat(acc[:]), flat(prev[:]),
                                             flat(tmp[:]))
                        started = True
                    else:
                        nc.gpsimd.tensor_add(flat(acc[:]), flat(acc[:]),
                                             flat(tmp[:]))

            def fused_stage(fw_tile, zt, pad, stride, interior, psum3, fl):
                for t in range(25):
                    ky, kx = divmod(t, 5)
                    y0 = interior - pad + stride * ky
                    x0 = interior - pad + stride * kx
                    for h in range(2):
                        s, e = fl(t, h)
                        nc.tensor.matmul(
                            psum3[:, 16 * h:16 * h + 16, :],
                            fw_tile[:, t, :],
                            zt[:, y0 + 16 * h:y0 + 16 * h + 16, x0:x0 + 32],
                            start=s, stop=e)

            class StpFlags:
                def __init__(self, total):
                    self.total = total
                    self.idx = [0, 0]

                def next(self, h):
                    i = self.idx[h]
                    self.idx[h] += 1
                    return (i == 0, i == self.total - 1)

            def preproc(b):
                states = states_t[b % 2]
                for inp, xd in ((0, x0_d), (1, x1_d)):
                    scr = pspool.tile([128, 32, 32], F32, tag="stp")
                    for kc in range(4):
                        xb = xpool.tile([128, HW], F32, tag="xb")
                        nc.sync.dma_start(xb[:], xd[b, kc])
                        xr = xpool.tile([128, HW], F32R, tag="xr")
                        nc.scalar.activation(xr[:], xb[:], ACTF.Relu)
                        for h in range(2):
                            nc.tensor.matmul(
                                scr[:, 16 * h:16 * (h + 1), :],
                                prew[:, inp, kc, :],
                                xr[:, 512 * h:512 * (h + 1)].rearrange(
                                    "p (a c) -> p a c", a=16),
                                start=(kc == 0), stop=(kc == 3))
                    nc.scalar.copy(states[:, inp], scr[:])

            def build_pools(j, b):
                states = states_t[b % 2]
                x3 = states[:, j]
                nc.scalar.copy(xpmax[:, 1:33, 1:33], x3)
                t = ptmp[0]
                nc.vector.tensor_max(t[:], xpmax[:, 1:33, 0:32],
                                     xpmax[:, 1:33, 1:33])
                nc.vector.tensor_max(rmpad[:, 1:33, :], t[:],
                                     xpmax[:, 1:33, 2:34])
                nc.vector.tensor_max(t[:], rmpad[:, 0:32, :],
                                     rmpad[:, 1:33, :])
                nc.vector.tensor_max(pooled[:, 0, j], t[:],
                                     rmpad[:, 2:34, :])
                nc.scalar.copy(xpsum[:, 1:33, 1:33], x3)
                t = ptmp[1]
                nc.gpsimd.tensor_add(t[:], xpsum[:, 1:33, 0:32],
                                     xpsum[:, 1:33, 1:33])
                nc.gpsimd.tensor_add(rspad[:, 1:33, :], t[:],
                                     xpsum[:, 1:33, 2:34])
                nc.gpsimd.tensor_add(t[:], rspad[:, 0:32, :],
                                     rspad[:, 1:33, :])
                nc.gpsimd.tensor_add(ptmp[0][:], t[:], rspad[:, 2:34, :])
                nc.gpsimd.tensor_mul(pooled[:, 1, j], ptmp[0][:], rmap[:])

            # ================= main =================
            preproc(0)
            for b in range(BL):
                states = states_t[b % 2]
                for step in range(STEPS):
                    n_in = 2 + step
                    m0 = OFFSETS[step]
                    stp = pspool.tile([128, 32, 32], F32, tag="stp")
                    total = 0
                    for j in range(n_in):
                        m = m0 + j
                        total += (25 if (m, b) not in DIL5_DVE else 1) + 25 + 1 + 1
                    fl = StpFlags(total)

                    pending = []

                    for j in range(n_in):
                        m = m0 + j
                        jb = j % 2
                        nc.scalar.activation(z36[jb][:, 2:34, 2:34],
                                             states[:, j], ACTF.Relu)
                        # sep5 stage 1 (PE) -> scr5
                        fw_a = fwpool.tile([128, 25, 128], BF16, tag="fw")
                        nc.sync.dma_start(fw_a[:], fw5a_d[:, m])
                        scr5 = scrpool.tile([128, 32, 32], F32, tag="scr")
                        fused_stage(fw_a, z36[jb], 2, 1, 2, scr5,
                                    lambda t, h: (t == 0, t == 24))
                        # sep3 stage 1 (DVE) -> scr3
                        da1 = dpool.tile([128, 32, 32], BF16, tag="dwacc")
                        dw_chain_dve(z36[jb], da1,
                                     lambda t: dwt[:, m, TAP_S3A + t:TAP_S3A + t + 1],
                                     3, 1, 1, 2)
                        scr3 = scrpool.tile([128, 32, 32], F32, tag="scr")
                        mm_chunks(scr3, pw[:, m, PW_S3A, :], da1,
                                  lambda h: (True, True))
                        # dil3 (ScalarE+GpSimd hybrid)
                        acc3 = dpool.tile([128, 32, 32], BF16, tag="dwacc")
                        dw_chain_hyb(z36[jb], acc3,
                                     lambda t: dwd3[:, m, b, t:t + 1],
                                     3, 2, 2, 2)
                        # dil5
                        nc.scalar.activation(z40[jb][:, 4:36, 4:36],
                                             states[:, j], ACTF.Relu,
                                             scale=alf[:, m, b, O_DIL5:O_DIL5 + 1])
                        if (m, b) not in DIL5_DVE:
                            fw_d = fwpool.tile([128, 25, 128], BF16, tag="fw")
                            nc.sync.dma_start(fw_d[:], fwd5_d[:, m])
                            fused_stage(fw_d, z40[jb], 4, 2, 4, stp,
                                        lambda t, h: fl.next(h))
                        else:
                            da5 = dpool.tile([128, 32, 32], BF16, tag="dwacc")
                            dw_chain_dve(z40[jb], da5,
                                         lambda t: dwt[:, m, TAP_D5 + t:TAP_D5 + t + 1],
                                         5, 4, 2, 4)
                            mm_chunks(stp, pw[:, m, PW_D5, :], da5,
                                      lambda h: fl.next(h))
                        # dil3 pw (late: gives the Sc+GpSimd chain time)
                        mm_chunks(stp, pw[:, m, PW_D3, :], acc3,
                                  lambda h: fl.next(h))

                        def make_stage2(m=m, jb=jb, scr5=scr5, scr3=scr3):
                            def emit():
                                nc.scalar.activation(
                                    z5b[jb][:, 2:34, 2:34], scr5[:], ACTF.Relu,
                                    scale=alf[:, m, b, O_SEP5:O_SEP5 + 1])
                                fw_b = fwpool.tile([128, 25, 128], BF16, tag="fw")
                                nc.sync.dma_start(fw_b[:], fw5b_d[:, m])
                                fused_stage(fw_b, z5b[jb], 2, 1, 2, stp,
                                            lambda t, h: fl.next(h))
                                nc.scalar.activation(
                                    z2b[jb][:, 2:34, 2:34], scr3[:], ACTF.Relu,
                                    scale=alf[:, m, b, O_SEP3:O_SEP3 + 1])
                                da2 = dpool.tile([128, 32, 32], BF16, tag="dwacc")
                                dw_chain_dve(z2b[jb], da2,
                                             lambda t: dwt[:, m, TAP_S3B + t:TAP_S3B + t + 1],
                                             3, 1, 1, 2)
                                mm_chunks(stp, pw[:, m, PW_S3B, :], da2,
                                          lambda h: fl.next(h))
                            return emit

                        pending.append(make_stage2())
                        if len(pending) > 1:
                            pending.pop(0)()

                    while pending:
                        pending.pop(0)()

                    if step == 0:
                        build_pools(0, b)
                        build_pools(1, b)
                    else:
                        build_pools(1 + step, b)

                    for j in range(n_in):
                        m = m0 + j
                        for (src, o) in ((pooled[:, 0, j], O_MAX),
                                         (pooled[:, 1, j], O_AVG),
                                         (states[:, j], O_SKIP)):
                            nc.vector.scalar_tensor_tensor(
                                stp[:], src, alf[:, m, b, o:o + 1], stp[:],
                                op0=ALU.mult, op1=ALU.add)

                    nc.scalar.copy(states[:, 2 + step], stp[:])

                    if step == STEPS - 1 and b + 1 < BL:
                        preproc(b + 1)

                for i in range(4):
                    nc.sync.dma_start(out_d[b, i], flat(states[:, 2 + i]))

    nc.compile()
    return nc


def host_prepare(inputs):
    s0, s1 = np.asarray(inputs["s0"]), np.asarray(inputs["s1"])
    gates = np.asarray(inputs["gates"])
    top = int(inputs["top"])
    p = _host_alphas(gates, top)

    prew = np.empty((128, 2, 4, 128), np.float32)
    for inp, wname in ((0, "pre0_w"), (1, "pre1_w")):
        wmat = np.asarray(inputs[wname]) * BN_SCALE
        for kc in range(4):
            prew[:, inp, kc, :] = wmat[:, 128 * kc:128 * (kc + 1)].T

    pw = np.empty((128, N_MIX, N_PW, 128), np.float32)
    for slot, key in ((PW_S3A, "sep3_pw1"), (PW_S3B, "sep3_pw2"),
                      (PW_D3, "dil3_pw"), (PW_D5, "dil5_pw")):
        wmat = np.asarray(inputs[key]).astype(np.float32) * BN_SCALE
        pw[:, :, slot, :] = wmat.transpose(2, 0, 1)

    def fuse(pw_key, dw_key):
        pwm = np.asarray(inputs[pw_key]).astype(np.float32) * BN_SCALE
        dwm = np.asarray(inputs[dw_key]).astype(np.float32).reshape(N_MIX, C, 25)
        pwT = pwm.transpose(2, 0, 1)
        dwT = dwm.transpose(1, 0, 2)
        return (pwT[:, :, None, :] * dwT[:, :, :, None]).astype(ml_dtypes.bfloat16)

    fw5a = fuse("sep5_pw1", "sep5_dw1")
    fw5b = fuse("sep5_pw2", "sep5_dw2")
    fwd5 = fuse("dil5_pw", "dil5_dw")

    dwt = np.empty((128, N_MIX, N_TAPS), np.float32)
    for t0, key, k in ((TAP_S3A, "sep3_dw1", 3), (TAP_S3B, "sep3_dw2", 3),
                       (TAP_D5, "dil5_dw", 5)):
        w = np.asarray(inputs[key])
        dwt[:, :, t0:t0 + k * k] = (
            w.reshape(N_MIX, C, k * k).transpose(1, 0, 2))

    cnt = np.zeros((32, 32), np.float32)
    for dy in (-1, 0, 1):
        for dx in (-1, 0, 1):
            cnt[max(0, dy):32 - max(0, -dy),
                max(0, dx):32 - max(0, -dx)] += 1
    rmap = np.broadcast_to((BN_SCALE / cnt).astype(np.float32),
                           (128, 32, 32)).copy()

    d3w = np.asarray(inputs["dil3_dw"]).astype(np.float32).reshape(N_MIX, C, 9)

    in_maps = []
    for core in range(N_CORES):
        sl = slice(core * BL, (core + 1) * BL)
        alf = p[:, sl, :].copy()
        alf[:, :, O_MAX] *= BN_SCALE
        alf_b = np.broadcast_to(alf, (128,) + alf.shape).copy()
        dwd3 = np.einsum('mct,mb->cmbt', d3w, p[:, sl, O_DIL3]).astype(np.float32)
        dwd3 = np.ascontiguousarray(dwd3)
        in_maps.append({
            "x0": s0[sl].reshape(BL, 4, 128, HW).astype(np.float32),
            "x1": s1[sl].reshape(BL, 4, 128, HW).astype(np.float32),
            "prew": prew,
            "pw": pw.astype(ml_dtypes.bfloat16),
            "dwt": dwt, "dwd3": dwd3,
            "fw5a": fw5a, "fw5b": fw5b, "fwd5": fwd5,
            "alf": alf_b.astype(np.float32), "rmap": rmap,
        })
    return in_maps, p


_prog_cache = {}


def _get_dense_program():
    if "v3" not in _prog_cache:
        _prog_cache["v3"] = build_program()
    return _prog_cache["v3"]


def kernel(**inputs):
    in_maps, _ = host_prepare(inputs)
    nc = _get_dense_program()
    res = run_bass_kernel_spmd(nc, in_maps, core_ids=list(range(N_CORES)))
    out = np.empty((B, 512, H, W), np.float32)
    for core in range(N_CORES):
        o = res.results[core]["out"]
        out[core * BL:(core + 1) * BL] = (
            o.reshape(BL, 512, H, W).astype(np.float32))
    return out


# revision 6
# speedup vs baseline: 1.2696x; 1.2659x over previous
"""DARTS-cell (moe_routing) Trainium2 kernel, v4.

Data-parallel over batch B=32 across 8 cores (4 samples/core).

Engine split (measured rates on TRN2):
- PE (~244 ns per N=512 matmul): preprocess, all pointwise convs, and
  PE-fused depthwise taps for sep5 (both stages), dil5 (most (m,b)),
  and dil3 (per-sample alpha-folded fused matrices).
- DVE: sep3 tap chains (tensor_scalar ~476 ns + STT ~1285 ns each; STT
  has no fast mode on cayman), dil5 chains for a few (m,b) to balance,
  pool maxes, pool/skip accumulation into the state PSUM.
- ScalarE: relu/pad builds, state evacuation. GpSimd: avg pools only
  (concurrent GpSimd streaming degrades DVE via SBUF port contention,
  so it gets nothing else).

Boundary pipelining: next step's first relu builds and the next
sample's preprocess are emitted ahead of the pool/post/evac tail so the
PE never drains at step/sample boundaries (HAM stays warm).
"""

import sys

sys.path.insert(0, "/opt/trn_rl_repo")

import numpy as np
import ml_dtypes
from concourse import bacc, mybir, tile
from concourse.bass_utils import run_bass_kernel_spmd

STEPS = 4
N_MIX = 14
OFFSETS = [0, 2, 5, 9]
B, C_IN, C, H, W = 32, 512, 128, 32, 32
HW = H * W
N_CORES = 8
BL = B // N_CORES
BN_SCALE = float(1.0 / np.sqrt(1.0 + 1e-5))

F32 = mybir.dt.float32
F32R = mybir.dt.float32r
BF16 = mybir.dt.bfloat16
ALU = mybir.AluOpType
ACTF = mybir.ActivationFunctionType

O_MAX, O_AVG, O_SKIP, O_SEP3, O_SEP5, O_DIL3, O_DIL5 = 1, 2, 3, 4, 5, 6, 7

TAP_S3A, TAP_S3B, TAP_D5 = 0, 9, 18
N_TAPS = 43
PW_S3A, PW_S3B, PW_D3, PW_D5 = 0, 1, 2, 3
N_PW = 4

# (m, b) pairs whose dil5 branch runs as a pure-DVE tap chain (PE relief)
DIL5_DVE_COUNT = 16
_pairs = [(m, b) for b in range(BL) for m in range(N_MIX)]
DIL5_DVE = set(_pairs[i] for i in
               np.linspace(0, len(_pairs) - 1, DIL5_DVE_COUNT).astype(int)) \
    if DIL5_DVE_COUNT else set()


def _host_alphas(gates, top):
    g = gates.astype(np.float64)
    idx = np.argsort(-g, axis=-1, kind="stable")[..., :top]
    mask = np.zeros(g.shape, bool)
    np.put_along_axis(mask, idx, True, axis=-1)
    gm = np.where(mask, g, -np.inf)
    gm -= gm.max(axis=-1, keepdims=True)
    e = np.exp(gm)
    p = e / e.sum(axis=-1, keepdims=True)
    return p.astype(np.float32)


def build_program(n_cores=N_CORES):
    nc = bacc.Bacc("TRN2", target_bir_lowering=False, debug=False,
                   num_devices=n_cores)

    x0_d = nc.dram_tensor("x0", [BL, 4, 128, HW], F32, kind="ExternalInput").ap()
    x1_d = nc.dram_tensor("x1", [BL, 4, 128, HW], F32, kind="ExternalInput").ap()
    prew_d = nc.dram_tensor("prew", [128, 2, 4, 128], F32R, kind="ExternalInput").ap()
    pw_d = nc.dram_tensor("pw", [128, N_MIX, N_PW, 128], BF16, kind="ExternalInput").ap()
    fw5a_d = nc.dram_tensor("fw5a", [128, N_MIX, 25, 128], BF16, kind="ExternalInput").ap()
    fw5b_d = nc.dram_tensor("fw5b", [128, N_MIX, 25, 128], BF16, kind="ExternalInput").ap()
    fwd5_d = nc.dram_tensor("fwd5", [128, N_MIX, 25, 128], BF16, kind="ExternalInput").ap()
    fwd3_d = nc.dram_tensor("fwd3", [128, N_MIX, BL, 9, 128], BF16, kind="ExternalInput").ap()
    dwt_d = nc.dram_tensor("dwt", [128, N_MIX, N_TAPS], F32, kind="ExternalInput").ap()
    alf_d = nc.dram_tensor("alf", [128, N_MIX, BL, 8], F32, kind="ExternalInput").ap()
    rmap_d = nc.dram_tensor("rmap", [128, 32, 32], F32, kind="ExternalInput").ap()
    out_d = nc.dram_tensor("out", [BL, 4, 128, HW], F32, kind="ExternalOutput").ap()

    with tile.TileContext(nc) as tc:
        with (
            tc.tile_pool(name="const", bufs=1) as cpool,
            tc.tile_pool(name="work", bufs=1) as wpool,
            tc.tile_pool(name="xs", bufs=2) as xpool,
            tc.tile_pool(name="dwa", bufs=4) as dpool,
            tc.tile_pool(name="ps_state", bufs=2, space="PSUM") as pspool,
            tc.tile_pool(name="ps_scr", bufs=2, space="PSUM") as scrpool,
            tc.tile_pool(name="fw", bufs=2) as fwpool,
            tc.tile_pool(name="fw3", bufs=2) as fw3pool,
        ):
            prew = cpool.tile([128, 2, 4, 128], F32R, tag="prew")
            pw = cpool.tile([128, N_MIX, N_PW, 128], BF16, tag="pw")
            dwt = cpool.tile([128, N_MIX, N_TAPS], F32, tag="dwt")
            alf = cpool.tile([128, N_MIX, BL, 8], F32, tag="alf")
            rmap = cpool.tile([128, 32, 32], F32, tag="rmap")
            nc.sync.dma_start(prew[:], prew_d)
            nc.sync.dma_start(pw[:], pw_d)
            nc.sync.dma_start(dwt[:], dwt_d)
            nc.sync.dma_start(alf[:], alf_d)
            nc.sync.dma_start(rmap[:], rmap_d)

            z36 = [wpool.tile([128, 36, 36], BF16, tag=f"z36_{i}", name=f"z36_{i}") for i in range(3)]
            z2b = [wpool.tile([128, 36, 36], BF16, tag=f"z2b_{i}", name=f"z2b_{i}") for i in range(2)]
            z5b = [wpool.tile([128, 36, 36], BF16, tag=f"z5b_{i}", name=f"z5b_{i}") for i in range(2)]
            z40 = [wpool.tile([128, 40, 40], BF16, tag=f"z40_{i}", name=f"z40_{i}") for i in range(2)]
            xpmax = wpool.tile([128, 34, 34], F32, tag="xpmax")
            xpsum = wpool.tile([128, 34, 34], F32, tag="xpsum")
            rmpad = wpool.tile([128, 34, 32], F32, tag="rmpad")
            rspad = wpool.tile([128, 34, 32], F32, tag="rspad")
            ptmp = [wpool.tile([128, 32, 32], F32, tag=f"ptmp_{i}", name=f"ptmp_{i}") for i in range(2)]

            states_t = [wpool.tile([128, 6, 32, 32], F32, tag=f"states_{i}", name=f"states_{i}") for i in range(2)]
            pooled = wpool.tile([128, 2, 5, 32, 32], BF16, tag="pooled")

            for z in z36 + z2b + z5b + z40:
                nc.gpsimd.memset(z[:], 0.0)
            nc.gpsimd.memset(xpmax[:], -1e30)
            nc.gpsimd.memset(xpsum[:], 0.0)
            nc.gpsimd.memset(rmpad[:], -1e30)
            nc.gpsimd.memset(rspad[:], 0.0)

            def flat(ap3):
                return ap3.rearrange("p a b -> p (a b)")

            def mm_chunks(psum3, lhsT, rhs3, fl):
                for h in range(2):
                    s, e = fl(h)
                    nc.tensor.matmul(psum3[:, 16 * h:16 * h + 16, :], lhsT,
                                     rhs3[:, 16 * h:16 * h + 16, :],
                                     start=s, stop=e)

            def tap_views(z, k, pad, stride, interior):
                out = []
                for ky in range(k):
                    for kx in range(k):
                        t = ky * k + kx
                        y0 = interior - pad + stride * ky
                        x0 = interior - pad + stride * kx
                        out.append((t, x0 % 2 == 0,
                                    z[:, y0:y0 + 32, x0:x0 + 32]))
                out.sort(key=lambda e: (not e[1],))
                return out

            def dw_chain_dve(z, dwacc, sc_of, k, pad, stride, interior):
                for i, (t, _, view) in enumerate(tap_views(z, k, pad, stride, interior)):
                    if i == 0:
                        nc.vector.tensor_scalar_mul(dwacc[:], view, sc_of(t))
                    else:
                        nc.vector.scalar_tensor_tensor(
                            dwacc[:], view, sc_of(t), dwacc[:],
                            op0=ALU.mult, op1=ALU.add)

            def fused_stage(fw_tile, zt, k, pad, stride, interior, psum3, fl):
                for t in range(k * k):
                    ky, kx = divmod(t, k)
                    y0 = interior - pad + stride * ky
                    x0 = interior - pad + stride * kx
                    for h in range(2):
                        s, e = fl(t, h)
                        nc.tensor.matmul(
                            psum3[:, 16 * h:16 * h + 16, :],
                            fw_tile[:, t, :],
                            zt[:, y0 + 16 * h:y0 + 16 * h + 16, x0:x0 + 32],
                            start=s, stop=e)

            class StpFlags:
                def __init__(self, total):
                    self.total = total
                    self.idx = [0, 0]

                def next(self, h):
                    i = self.idx[h]
                    self.idx[h] += 1
                    return (i == 0, i == self.total - 1)

            def preproc(b):
                states = states_t[b % 2]
                for inp, xd in ((0, x0_d), (1, x1_d)):
                    scr = scrpool.tile([128, 32, 32], F32, tag="scr")
                    for kc in range(4):
                        xb = xpool.tile([128, HW], F32, tag="xb")
                        nc.sync.dma_start(xb[:], xd[b, kc])
                        xr = xpool.tile([128, HW], F32R, tag="xr")
                        nc.scalar.activation(xr[:], xb[:], ACTF.Relu)
                        for h in range(2):
                            nc.tensor.matmul(
                                scr[:, 16 * h:16 * (h + 1), :],
                                prew[:, inp, kc, :],
                                xr[:, 512 * h:512 * (h + 1)].rearrange(
                                    "p (a c) -> p a c", a=16),
                                start=(kc == 0), stop=(kc == 3))
                    nc.scalar.copy(states[:, inp], scr[:])

            def build_z36(j, b):
                nc.scalar.activation(z36[j % 3][:, 2:34, 2:34],
                                     states_t[b % 2][:, j], ACTF.Relu)

            def build_pools(j, b):
                states = states_t[b % 2]
                x3 = states[:, j]
                nc.scalar.copy(xpmax[:, 1:33, 1:33], x3)
                t = ptmp[0]
                nc.vector.tensor_max(t[:], xpmax[:, 1:33, 0:32],
                                     xpmax[:, 1:33, 1:33])
                nc.vector.tensor_max(rmpad[:, 1:33, :], t[:],
                                     xpmax[:, 1:33, 2:34])
                nc.vector.tensor_max(t[:], rmpad[:, 0:32, :],
                                     rmpad[:, 1:33, :])
                nc.vector.tensor_max(pooled[:, 0, j], t[:],
                                     rmpad[:, 2:34, :])
                nc.scalar.copy(xpsum[:, 1:33, 1:33], x3)
                t = ptmp[1]
                nc.gpsimd.tensor_add(t[:], xpsum[:, 1:33, 0:32],
                                     xpsum[:, 1:33, 1:33])
                nc.gpsimd.tensor_add(rspad[:, 1:33, :], t[:],
                                     xpsum[:, 1:33, 2:34])
                nc.gpsimd.tensor_add(t[:], rspad[:, 0:32, :],
                                     rspad[:, 1:33, :])
                nc.gpsimd.tensor_add(ptmp[0][:], t[:], rspad[:, 2:34, :])
                nc.gpsimd.tensor_mul(pooled[:, 1, j], ptmp[0][:], rmap[:])

            # ================= main =================
            preproc(0)
            build_z36(0, 0)
            build_z36(1, 0)
            for b in range(BL):
                states = states_t[b % 2]
                for step in range(STEPS):
                    n_in = 2 + step
                    m0 = OFFSETS[step]
                    stp = pspool.tile([128, 32, 32], F32, tag="stp")
                    total = 0
                    for j in range(n_in):
                        m = m0 + j
                        total += (25 if (m, b) not in DIL5_DVE else 1) + 25 + 9 + 1
                    fl = StpFlags(total)

                    pending = []

                    for j in range(n_in):
                        m = m0 + j
                        jb = j % 3
                        if j >= 2:  # 0,1 hoisted by the previous tail
                            build_z36(j, b)
                        # sep5 stage 1 (PE) -> scr5
                        fw_a = fwpool.tile([128, 25, 128], BF16, tag="fw")
                        nc.sync.dma_start(fw_a[:], fw5a_d[:, m])
                        scr5 = scrpool.tile([128, 32, 32], F32, tag="scr")
                        fused_stage(fw_a, z36[jb], 5, 2, 1, 2, scr5,
                                    lambda t, h: (t == 0, t == 24))
                        # sep3 stage 1 (DVE) -> scr3
                        da1 = dpool.tile([128, 32, 32], BF16, tag="dwacc")
                        dw_chain_dve(z36[jb], da1,
                                     lambda t: dwt[:, m, TAP_S3A + t:TAP_S3A + t + 1],
                                     3, 1, 1, 2)
                        scr3 = scrpool.tile([128, 32, 32], F32, tag="scr")
                        mm_chunks(scr3, pw[:, m, PW_S3A, :], da1,
                                  lambda h: (True, True))
                        # dil3 (PE-fused, per-sample alpha-folded weights)
                        fw_3 = fw3pool.tile([128, 9, 128], BF16, tag="fw3")
                        nc.sync.dma_start(fw_3[:], fwd3_d[:, m, b])
                        fused_stage(fw_3, z36[jb], 3, 2, 2, 2, stp,
                                    lambda t, h: fl.next(h))
                        # dil5
                        nc.scalar.activation(z40[j % 2][:, 4:36, 4:36],
                                             states[:, j], ACTF.Relu,
                                             scale=alf[:, m, b, O_DIL5:O_DIL5 + 1])
                        if (m, b) not in DIL5_DVE:
                            fw_d = fwpool.tile([128, 25, 128], BF16, tag="fw")
                            nc.sync.dma_start(fw_d[:], fwd5_d[:, m])
                            fused_stage(fw_d, z40[j % 2], 5, 4, 2, 4, stp,
                                        lambda t, h: fl.next(h))
                        else:
                            da5 = dpool.tile([128, 32, 32], BF16, tag="dwacc")
                            dw_chain_dve(z40[j % 2], da5,
                                         lambda t: dwt[:, m, TAP_D5 + t:TAP_D5 + t + 1],
                                         5, 4, 2, 4)
                            mm_chunks(stp, pw[:, m, PW_D5, :], da5,
                                      lambda h: fl.next(h))

                        def make_stage2(m=m, jb=jb, scr5=scr5, scr3=scr3):
                            def emit():
                                nc.scalar.activation(
                                    z5b[jb % 2][:, 2:34, 2:34], scr5[:], ACTF.Relu,
                                    scale=alf[:, m, b, O_SEP5:O_SEP5 + 1])
                                fw_b = fwpool.tile([128, 25, 128], BF16, tag="fw")
                                nc.sync.dma_start(fw_b[:], fw5b_d[:, m])
                                fused_stage(fw_b, z5b[jb % 2], 5, 2, 1, 2, stp,
                                            lambda t, h: fl.next(h))
                                nc.scalar.activation(
                                    z2b[jb % 2][:, 2:34, 2:34], scr3[:], ACTF.Relu,
                                    scale=alf[:, m, b, O_SEP3:O_SEP3 + 1])
                                da2 = dpool.tile([128, 32, 32], BF16, tag="dwacc")
                                dw_chain_dve(z2b[jb % 2], da2,
                                             lambda t: dwt[:, m, TAP_S3B + t:TAP_S3B + t + 1],
                                             3, 1, 1, 2)
                                mm_chunks(stp, pw[:, m, PW_S3B, :], da2,
                                          lambda h: fl.next(h))
                            return emit

                        pending.append(make_stage2())
                        if len(pending) > 1:
                            pending.pop(0)()

                    while pending:
                        pending.pop(0)()

                    # ---- boundary hoists: keep PE fed past the tail ----
                    if step == STEPS - 1:
                        if b + 1 < BL:
                            preproc(b + 1)
                            build_z36(0, b + 1)
                            build_z36(1, b + 1)
                    else:
                        # next step's j=0,1 read z36 bufs 0,1; rebuild them
                        # ahead of the pool/post/evac tail (buffers may have
                        # been recycled by j>=3 of this step)
                        build_z36(0, b)
                        build_z36(1, b)

                    if step == 0:
                        build_pools(0, b)
                        build_pools(1, b)
                    else:
                        build_pools(1 + step, b)

                    for j in range(n_in):
                        m = m0 + j
                        for (src, o) in ((pooled[:, 0, j], O_MAX),
                                         (pooled[:, 1, j], O_AVG),
                                         (states[:, j], O_SKIP)):
                            nc.vector.scalar_tensor_tensor(
                                stp[:], src, alf[:, m, b, o:o + 1], stp[:],
                                op0=ALU.mult, op1=ALU.add)

                    nc.scalar.copy(states[:, 2 + step], stp[:])

                for i in range(4):
                    nc.sync.dma_start(out_d[b, i], flat(states[:, 2 + i]))

    nc.compile()
    return nc


def host_prepare(inputs):
    s0, s1 = np.asarray(inputs["s0"]), np.asarray(inputs["s1"])
    gates = np.asarray(inputs["gates"])
    top = int(inputs["top"])
    p = _host_alphas(gates, top)

    prew = np.empty((128, 2, 4, 128), np.float32)
    for inp, wname in ((0, "pre0_w"), (1, "pre1_w")):
        wmat = np.asarray(inputs[wname]) * BN_SCALE
        for kc in range(4):
            prew[:, inp, kc, :] = wmat[:, 128 * kc:128 * (kc + 1)].T

    pw = np.empty((128, N_MIX, N_PW, 128), np.float32)
    for slot, key in ((PW_S3A, "sep3_pw1"), (PW_S3B, "sep3_pw2"),
                      (PW_D3, "dil3_pw"), (PW_D5, "dil5_pw")):
        wmat = np.asarray(inputs[key]).astype(np.float32) * BN_SCALE
        pw[:, :, slot, :] = wmat.transpose(2, 0, 1)

    def fuse(pw_key, dw_key, k=5):
        pwm = np.asarray(inputs[pw_key]).astype(np.float32) * BN_SCALE
        dwm = np.asarray(inputs[dw_key]).astype(np.float32).reshape(N_MIX, C, k * k)
        pwT = pwm.transpose(2, 0, 1)
        dwT = dwm.transpose(1, 0, 2)
        return (pwT[:, :, None, :] * dwT[:, :, :, None]).astype(ml_dtypes.bfloat16)

    fw5a = fuse("sep5_pw1", "sep5_dw1")
    fw5b = fuse("sep5_pw2", "sep5_dw2")
    fwd5 = fuse("dil5_pw", "dil5_dw")
    fwd3_base = fuse("dil3_pw", "dil3_dw", k=3)  # [128, M, 9, 128] bf16

    dwt = np.empty((128, N_MIX, N_TAPS), np.float32)
    for t0, key, k in ((TAP_S3A, "sep3_dw1", 3), (TAP_S3B, "sep3_dw2", 3),
                       (TAP_D5, "dil5_dw", 5)):
        w = np.asarray(inputs[key])
        dwt[:, :, t0:t0 + k * k] = (
            w.reshape(N_MIX, C, k * k).transpose(1, 0, 2))

    cnt = np.zeros((32, 32), np.float32)
    for dy in (-1, 0, 1):
        for dx in (-1, 0, 1):
            cnt[max(0, dy):32 - max(0, -dy),
                max(0, dx):32 - max(0, -dx)] += 1
    rmap = np.broadcast_to((BN_SCALE / cnt).astype(np.float32),
                           (128, 32, 32)).copy()

    fwd3f = fwd3_base.astype(np.float32)  # [128, M, 9, 128]

    in_maps = []
    for core in range(N_CORES):
        sl = slice(core * BL, (core + 1) * BL)
        alf = p[:, sl, :].copy()
        alf[:, :, O_MAX] *= BN_SCALE
        alf_b = np.broadcast_to(alf, (128,) + alf.shape).copy()
        # dil3 fused with per-sample alpha: [128, M, BL, 9, 128]
        fwd3 = (fwd3f[:, :, None, :, :] *
                p[None, :, sl, O_DIL3, None, None]).astype(ml_dtypes.bfloat16)
        in_maps.append({
            "x0": s0[sl].reshape(BL, 4, 128, HW).astype(np.float32),
            "x1": s1[sl].reshape(BL, 4, 128, HW).astype(np.float32),
            "prew": prew,
            "pw": pw.astype(ml_dtypes.bfloat16),
            "dwt": dwt,
            "fw5a": fw5a, "fw5b": fw5b, "fwd5": fwd5,
            "fwd3": np.ascontiguousarray(fwd3),
            "alf": alf_b.astype(np.float32), "rmap": rmap,
        })
    return in_maps, p


_prog_cache = {}


def _get_dense_program():
    if "v4" not in _prog_cache:
        _prog_cache["v4"] = build_program()
    return _prog_cache["v4"]


def kernel(**inputs):
    in_maps, _ = host_prepare(inputs)
    nc = _get_dense_program()
    res = run_bass_kernel_spmd(nc, in_maps, core_ids=list(range(N_CORES)))
    out = np.empty((B, 512, H, W), np.float32)
    for core in range(N_CORES):
        o = res.results[core]["out"]
        out[core * BL:(core + 1) * BL] = (
            o.reshape(BL, 512, H, W).astype(np.float32))
    return out
